# revision 28
# baseline (speedup 1.0000x reference)
"""Multi-head causal attention (B=2, T=2048, D=1024, H=16) on 8 TRN2 cores.

Sharding: core c handles batch b=c//4 and head group g=c%4 (4 heads each).
Device kernel per core (fp32r matmuls, fp16 attention weights):
  QT/KT = (X Wq/Wk)^T in [head*64, T] layout
  V     = X Wv in [T, 256] layout fp16 with a ones-column per head (V_aug)
  per q-tile (512 q), per head pair, per key-block (128 k, causal):
    ST[k,q] = KT-vs-QT matmul (PSUM, 2 heads row-packed on the PE array)
    PT = exp(ST/8) fp16 (no max subtraction; S/8 ~ N(0,1))
    diag blocks: zero PT where k > q (DVE affine_select)
    CT[65,512] += V_aug-vs-PT matmul (row 64 = softmax denominators l)
  CT_norm = CT[0:64] * (1/l broadcast via gpsimd partition_broadcast)
  OT[d, q-tile] = WO-vs-CT_norm matmul -> fp16 partial, DMA'd per q-tile
Host: out[b] = sum over the 4 head-group cores of OT^T, + bo.

The qt-outer loop pipelines projection slices (n=qt+1) and output
projection (n=qt) into the ACT-bound attention phase.
"""

import numpy as np

B, T, D, H, HD = 2, 2048, 1024, 16, 64
NCORES = 8
HPC = 4          # heads per core
GC = HPC * HD    # 256 columns per head group
NQ = 512         # q-tile width
KB = 128         # key block

_cache = {}


def _build(t_len):
    from concourse import bacc
    import concourse.tile as tile
    import concourse.mybir as mybir

    dt = mybir.dt
    f32, f32r, f16 = dt.float32, dt.float32r, dt.float16

    n_qt = t_len // NQ           # q tiles (and 512-wide n tiles)
    n_tt = t_len // KB           # token tiles of 128
    n_kc = D // KB               # contraction chunks over D (8)

    nc = bacc.Bacc("TRN2", debug=False, num_devices=NCORES)

    xt = nc.dram_tensor("XT", [D, t_len], f16, kind="ExternalInput")
    wq = nc.dram_tensor("WQ", [D, GC], f16, kind="ExternalInput")
    wk = nc.dram_tensor("WK", [D, GC], f16, kind="ExternalInput")
    wv = nc.dram_tensor("WV", [D, GC], f16, kind="ExternalInput")
    wo = nc.dram_tensor("WO", [GC, D], f16, kind="ExternalInput")
    ot = nc.dram_tensor("OT", [D, t_len], f16, kind="ExternalOutput")

    with tile.TileContext(nc) as tc:
        with (
            tc.tile_pool(name="w", bufs=1) as wpool,
            tc.tile_pool(name="proj", bufs=1) as proj,
            tc.tile_pool(name="pt", bufs=8) as ptpool,
            tc.tile_pool(name="small", bufs=6) as small,
            tc.tile_pool(name="ot", bufs=8) as otpool,
            tc.tile_pool(name="ps", bufs=2, space="PSUM") as pspool,
            tc.tile_pool(name="st", bufs=2, space="PSUM") as stpool,
            tc.tile_pool(name="ct", bufs=1, space="PSUM") as ctpool,
        ):
            # ---- load inputs (weights first so matmuls can start early) ----
            w_sb = {}

            def load_w(name, dram):
                t_ = wpool.tile([128, n_kc, GC], f16, tag=f"w{name}",
                                name=f"w{name}")
                nc.sync.dma_start(
                    out=t_, in_=dram.ap().rearrange("(c p) n -> p c n", p=128)
                )
                w_sb[name] = t_

            xt_sb = []

            def load_xt(c):
                t_ = wpool.tile([128, t_len], f16, tag=f"xt{c}",
                                name=f"xt{c}")
                nc.sync.dma_start(out=t_, in_=xt.ap()[c * 128:(c + 1) * 128, :])
                xt_sb.append(t_)

            load_xt(0)
            load_w("q", wq)
            load_xt(1)
            load_w("k", wk)
            for c in range(2, n_kc):
                load_xt(c)
            load_w("v", wv)
            wo_sb = wpool.tile([128, 2, D], f16, tag="wo")
            nc.sync.dma_start(
                out=wo_sb, in_=wo.ap().rearrange("(c p) n -> p c n", p=128)
            )

            qt_sb = [proj.tile([128, t_len], f32r, tag=f"qt{m}", name=f"qt{m}")
                     for m in range(2)]
            kt_sb = [proj.tile([128, t_len], f32r, tag=f"kt{m}", name=f"kt{m}")
                     for m in range(2)]
            v_sb = proj.tile([128, n_tt, HPC, HD + 1], f16, tag="v")
            nc.vector.memset(v_sb, 1.0)
            ct_sb = [proj.tile([128, t_len], f16, tag=f"ct{m}", name=f"ctn{m}")
                     for m in range(2)]
            diag_mask = proj.tile([128, 2, NQ], f16, tag="dmask")
            nc.vector.memset(diag_mask, 1.0)
            nc.gpsimd.affine_select(
                out=diag_mask,
                in_=diag_mask,
                compare_op=mybir.AluOpType.is_ge,
                fill=0.0,
                base=0,
                pattern=[[0, 2], [1, NQ]],
                channel_multiplier=-1,
            )

            def proj_slice(n, skip_qk=False):
                """Project QT/KT n-slice (both m tiles) + V token tiles."""
                for dst, wname in () if skip_qk else ((qt_sb, "q"), (kt_sb, "k")):
                    for m in range(2):
                        ps = pspool.tile([128, NQ], f32, tag="ps", name="ps")
                        for kc in range(n_kc):
                            nc.tensor.matmul(
                                ps,
                                w_sb[wname][:, kc, m * 128:(m + 1) * 128],
                                xt_sb[kc][:, n * NQ:(n + 1) * NQ],
                                start=(kc == 0),
                                stop=(kc == n_kc - 1),
                            )
                        nc.vector.tensor_copy(
                            dst[m][:, n * NQ:(n + 1) * NQ], ps
                        )
                for tt in range(4 * n, 4 * n + 4):
                    ps = pspool.tile([128, NQ], f32, tag="ps", name="ps")
                    for kc in range(n_kc):
                        nc.tensor.matmul(
                            ps[:, 0:GC],
                            xt_sb[kc][:, tt * 128:(tt + 1) * 128],
                            w_sb["v"][:, kc, :],
                            start=(kc == 0),
                            stop=(kc == n_kc - 1),
                        )
                    nc.vector.tensor_copy(
                        v_sb[:, tt, :, 0:HD],
                        ps[:, 0:GC].rearrange("p (h d) -> p h d", h=HPC),
                    )

            def attention(qt, mid=None):
                q0 = qt * NQ
                nkb = (q0 + NQ) // KB
                for hp in range(2):
                    if hp == 1 and mid is not None:
                        mid()
                    ct_ps = [
                        ctpool.tile([HD + 1, NQ], f32, tag=f"ct{i}",
                                    name=f"ctps{i}")
                        for i in range(2)
                    ]
                    for kb in range(nkb):
                        k0 = kb * KB
                        off = max(0, k0 - q0)   # fully-masked q prefix
                        w = NQ - off
                        diag = k0 + KB > q0
                        st = stpool.tile([128, 2 * NQ], f32, name="st")
                        st3 = st.rearrange("p (i q) -> p i q", i=2)
                        for i in range(2):
                            # fp32r is 4x slower below 256 moving cols; use
                            # the full width when the narrowed slice is small
                            s_off = off if w >= 256 else 0
                            nc.tensor.matmul(
                                st3[:, i, s_off:],
                                kt_sb[hp][i * 64:(i + 1) * 64, k0:k0 + KB],
                                qt_sb[hp][i * 64:(i + 1) * 64,
                                          q0 + s_off:q0 + NQ],
                                start=True,
                                stop=True,
                                tile_position=(64 * i, 0),
                            )
                        pt = ptpool.tile([128, 2 * NQ], f16, name="pt")
                        pt3 = pt.rearrange("p (i q) -> p i q", i=2)
                        nc.scalar.activation(
                            out=pt3[:, :, off:], in_=st3[:, :, off:],
                            func=mybir.ActivationFunctionType.Exp,
                            scale=float(1.0 / np.sqrt(HD)),
                        )
                        if diag:  # zero where k > q (x < p in slice coords)
                            nc.vector.tensor_mul(
                                pt3[:, :, off:],
                                pt3[:, :, off:],
                                diag_mask[:, :, 0:w],
                            )
                        for i in range(2):
                            nc.tensor.matmul(
                                ct_ps[i][:, off:],
                                v_sb[:, kb, 2 * hp + i, :],
                                pt3[:, i, off:],
                                start=(kb == 0),
                                stop=(kb == nkb - 1),
                            )
                    # normalize: ct_norm = ct[0:64] * (1/l), l = ct row 64.
                    # Stage to SBUF quickly so the PSUM bank frees early.
                    for i in range(2):
                        stage = small.tile([HD, NQ], f32, tag="stage",
                                           name="stage")
                        nc.vector.tensor_copy(stage, ct_ps[i][0:HD, :])
                        l_sb = small.tile([1, NQ], f32, tag="l_sb",
                                          name="l_sb")
                        nc.scalar.copy(l_sb, ct_ps[i][HD:HD + 1, :])
                        lb = small.tile([HD, NQ], f32, tag="lb", name="lb")
                        nc.gpsimd.partition_broadcast(lb, l_sb)
                        rb = small.tile([HD, NQ], f32, tag="rb", name="rb")
                        nc.vector.reciprocal(out=rb, in_=lb)
                        nc.vector.tensor_mul(
                            ct_sb[hp][i * 64:(i + 1) * 64, q0:q0 + NQ],
                            stage,
                            rb,
                        )

            def out_proj(n):
                for m in range(n_kc):  # 8 dout tiles of 128
                    ps = pspool.tile([128, NQ], f32, tag="ps", name="ps")
                    for cc in range(2):
                        nc.tensor.matmul(
                            ps,
                            wo_sb[:, cc, m * 128:(m + 1) * 128],
                            ct_sb[cc][:, n * NQ:(n + 1) * NQ],
                            start=(cc == 0),
                            stop=(cc == 1),
                        )
                    o_sb = otpool.tile([128, NQ], f16, name="o_sb")
                    nc.vector.tensor_copy(o_sb, ps)
                    nc.sync.dma_start(
                        out=ot.ap()[m * 128:(m + 1) * 128,
                                    n * NQ:(n + 1) * NQ],
                        in_=o_sb,
                    )

            def proj_first_qk():
                """QT/KT n=0 via the idle st-pool banks: 4 accumulation
                groups in flight so the PE tracks XT chunk arrivals."""
                st_a = stpool.tile([128, 2 * NQ], f32, name="st")
                st_b = stpool.tile([128, 2 * NQ], f32, name="st")
                regions = [
                    (qt_sb, "q", 0, st_a[:, 0:NQ]),
                    (qt_sb, "q", 1, st_a[:, NQ:2 * NQ]),
                    (kt_sb, "k", 0, st_b[:, 0:NQ]),
                    (kt_sb, "k", 1, st_b[:, NQ:2 * NQ]),
                ]
                for kc in range(n_kc):
                    for dst, wname, m, reg in regions:
                        nc.tensor.matmul(
                            reg,
                            w_sb[wname][:, kc, m * 128:(m + 1) * 128],
                            xt_sb[kc][:, 0:NQ],
                            start=(kc == 0),
                            stop=(kc == n_kc - 1),
                        )
                for dst, wname, m, reg in regions:
                    nc.vector.tensor_copy(dst[m][:, 0:NQ], reg)

            # ---- pipelined schedule ----
            proj_first_qk()
            proj_slice(0, skip_qk=True)
            for qt in range(n_qt):
                if qt == n_qt - 1 and n_qt > 1:
                    attention(qt, mid=lambda: out_proj(0))
                    continue_emit = True
                else:
                    attention(qt)
                if qt + 1 < n_qt:
                    proj_slice(qt + 1)
                else:
                    for n in range(1 if n_qt > 1 else 0, n_qt):
                        out_proj(n)

    nc.compile()
    return nc


def get_nc(t_len=T):
    if t_len not in _cache:
        _cache[t_len] = _build(t_len)
    return _cache[t_len]


def make_in_maps(X, Wq, Wk, Wv, Wo):
    X = np.asarray(X, dtype=np.float32)
    Wq = np.asarray(Wq, dtype=np.float32)
    Wk = np.asarray(Wk, dtype=np.float32)
    Wv = np.asarray(Wv, dtype=np.float32)
    Wo = np.asarray(Wo, dtype=np.float32)
    in_maps = []
    for c in range(NCORES):
        b, g = divmod(c, 4)
        cols = slice(g * GC, (g + 1) * GC)
        in_maps.append({
            "XT": np.ascontiguousarray(X[b].T).astype(np.float16),
            "WQ": np.ascontiguousarray(Wq[:, cols]).astype(np.float16),
            "WK": np.ascontiguousarray(Wk[:, cols]).astype(np.float16),
            "WV": np.ascontiguousarray(Wv[:, cols]).astype(np.float16),
            "WO": np.ascontiguousarray(Wo[cols, :]).astype(np.float16),
        })
    return in_maps


def gather_out(results, bo):
    out = np.zeros((B, T, D), dtype=np.float32)
    for c in range(NCORES):
        out[c // 4] += results[c]["OT"].T.astype(np.float32)
    out += np.asarray(bo, dtype=np.float32)
    return out


def kernel(X, Wq, Wk, Wv, Wo, bo):
    from concourse import bass_utils

    nc = get_nc(T)
    in_maps = make_in_maps(X, Wq, Wk, Wv, Wo)
    res = bass_utils.run_bass_kernel_spmd(
        nc, in_maps, core_ids=list(range(NCORES))
    )
    return gather_out(res.results, bo)
